# revision 1
# baseline (speedup 1.0000x reference)
"""Barlow Twins diagonal loss kernel for Trainium2 (8 NeuronCores).

Strategy
--------
Data-parallel over the batch dim: each of the 8 cores gets a 8192x512
shard of e and tau.  On-device, each core computes the five per-feature
batch reductions (sum_e, sum_tau, sum_e2, sum_tau2, sum_etau) with the
TensorEngine only:

  * inputs are cast f32 -> fp8e4m3 during the HBM->SBUF DMA (SWDGE
    cast) -- the 16 DMA engines each sustain ~26 GB/s on the f32 read
    side of the cast path, ~420-430 GB/s/core aggregate,
  * for each 128-row batch sub-tile and each 128-feature chunk c, five
    matmuls accumulate into one PSUM bank [128, 386] (f32):
       - e_c.T @ e_c   -> cols   0:128  (diag = sum_e2)
       - e_c.T @ t_c   -> cols 128:256  (diag = sum_etau)
       - e_c.T @ ones  -> col  384      (sum_e)
       - t_c.T @ t_c   -> cols 256:384  (diag = sum_tau2)
       - t_c.T @ ones  -> col  385      (sum_tau)
    (Plain fp8 mode: the PE streams rhs columns at ~0.4-0.7 ns/col, so
    a sub-tile costs ~1.05 us vs ~1.22 us of DMA -- the PE hides under
    the DMA stream.  Measured dead ends: MatmulPerfMode.DoubleRow is
    SLOWER per batch row on hardware (2 rhs column-slots per output
    column at the same column rate, plus 2.4x per-instruction
    overhead); fusing ee+et into one matmul via an [e|t]-paired tile
    reproducibly inflated DMA engine E79's per-packet time 25%.)
  * PSUM accumulates across all sub-tiles; one [128, 4, 386] f16 stats
    tensor is written back per core (f16 halves the drain bytes; the
    stats are O(8e3) sums with ~2e-4 f16 quantization, 100x inside the
    loss error budget).  Copies split vector/scalar into one SBUF tile
    that leaves as a single contiguous store DMA.

All 8 SWDGE semaphore lanes are used so a load's issue only waits on
the load 8 positions back (4 megas) -- deep enough that the DMA
engines never idle between megas or in the tapered tail (measured:
engines ~95% duty over the stream window, 420-433 GB/s sustained).

The host extracts the Gram diagonals, all-reduces the 8 partial stats
in float64 and evaluates the closed-form diagonal loss.  All precision-
critical accumulation happens in f32 PSUM; fp8 only quantizes the
individual products, which perturbs the final loss by ~1e-5 relative.
"""

import sys

if "/opt/trn_rl_repo" not in sys.path:
    sys.path.insert(0, "/opt/trn_rl_repo")

import numpy as np

N_CORES = 8
B, D = 65536, 512
BS = B // N_CORES  # 8192 rows per core
P = 128            # SBUF partitions / matmul contraction dim
CH = 128           # features per chunk (stationary width)
N_CH = D // CH     # 4 chunks
SW = 3 * CH + 2    # stats width per chunk: 3 Gram blocks + 2 sum columns
EPS = 1e-9

# mega-load schedule, in 128-row sub-tiles per mega-load (must sum to
# BS/128 = 64).  The PE consumes a mega only after its load fully
# completes, and per-sub-tile DMA time (~1.16us) barely exceeds PE
# time (~1.09us), so the critical path is max over megas of
# (load-complete + ALL remaining PE work).  A ts=8 mega ending at
# sub-tile C anchors at ~87.5 + 0.25*C us; the stream-end anchor sits
# at ~96.4us; equating them puts the big->small transition at C~32.
# ts=4 megas cost ~3.5% DMA rate (2KB packets vs 4KB), so earlier
# tapering than this loses more on the stream than it gains on the
# backlog (measured anchor table in the 2026-08-08 session).
MEGA_SCHED = [8, 8, 8, 8, 4, 4, 4, 4, 4, 4, 2, 2, 2, 1, 1]
N_LANES = 8                # SWDGE sem lanes (hw max): issue chain depth 8

TRACE = False              # test.py flips this to profile
LAST_RESULT = None         # BassKernelResults of the last run

_nc_cache = {}


def _build(bs=BS, sched=None):
    import concourse.bass as bass
    import concourse.tile as tile
    import concourse.tile_sem_assignment as tsa
    from concourse import mybir

    # Use all 8 SWDGE semaphore lanes: every instruction in this walrus
    # build has a single sync-wait slot, so each load DMA may carry at most
    # one lane-order wait; with 8 lanes the wait reaches 8 loads back (4
    # megas), which has always long completed -- loads issue stall-free.
    tsa.NUM_SWDGE_GLOBAL_SEMS = N_LANES

    from concourse.vector_clock import ScopedClock, VectorClock

    class _SplitDrainTC(tile.TileContext):
        """This walrus build rejects any instruction carrying more than ONE
        sync wait.  Tile's stock kernel-tail drain waits once per live proc
        lane on a single Drain instruction.  Replace it with one sync-engine
        nop per live lane (1 wait each, executed in program order on the SP
        sequencer) followed by a wait-free drain."""

        def _drain_and_barrier(self, tick_clock, wait_clock):
            gc = tick_clock.global_clock
            n = len(gc)
            for i in range(n):
                if gc[i] > 0:
                    vc = VectorClock([0] * n)
                    vc.require_at_least(i, gc[i])
                    nop = self.nc.sync.nop(nofuse=True)
                    wait_clock.add_sem_waits(nop.ins, ScopedClock({None: vc}))
            self.nc.sync.drain()
            self.nc.all_engine_barrier()
            assert self.sems is not None
            popped = self.nc._tile_sem_poison_stack.pop()
            assert popped is self._sem_poison
            self.nc.clear_and_free_semaphores(
                list(self.sems.allocated().values())
            )
            # stock Tile ends with a second all_engine_barrier here; it only
            # orders the range-clear above against the engines' exit-time
            # sem zeroing (idempotent zero writes over disjoint-or-zeroed
            # ranges) -- dropping it saves its ~0.5us rendezvous at the
            # measured end of the kernel.

    if sched is None:
        sched = list(MEGA_SCHED)
    assert sum(sched) * P == bs

    nc = bass.Bass()
    e = nc.dram_tensor("e", [bs, D], mybir.dt.float32, kind="ExternalInput")
    t = nc.dram_tensor("tau", [bs, D], mybir.dt.float32, kind="ExternalInput")
    # partition-major layout so the four chunk banks leave in ONE DMA of
    # contiguous 3088B-per-partition packets
    stats = nc.dram_tensor(
        "stats", [P, N_CH, SW], mybir.dt.float16, kind="ExternalOutput"
    )

    with _SplitDrainTC(nc) as tc:
        with (
            # every mega gets its own uniquely-tagged tiles (bufs=1, no slot
            # reuse) so no load DMA ever carries a WAW/WAR semaphore wait
            # (the direct2d DMA form only has one wait slot).
            tc.tile_pool(name="loads", bufs=1) as loads,
            tc.tile_pool(name="consts", bufs=1) as consts,
            tc.tile_pool(name="accs", bufs=1, space="PSUM") as accs,
            tc.tile_pool(name="outs", bufs=1) as outs,
        ):
            ones = consts.tile([P, 1], mybir.dt.float8e4)
            nc.vector.memset(ones, 1.0)
            # k-tiled ones for DoubleRow sum-matmuls (256 rows / instruction)
            ones2 = consts.tile([P, 2, 1], mybir.dt.float8e4)
            nc.vector.memset(ones2, 1.0)

            psums = [
                accs.tile([P, SW], mybir.dt.float32, name=f"acc{c}", tag=f"acc{c}")
                for c in range(N_CH)
            ]

            n_mega = len(sched)
            row0 = 0
            for m, ts_m in enumerate(sched):
                half = ts_m * D
                # row r = row0 + p*ts_m + s -> partition p, sub-tile s
                e_v = e[row0 : row0 + P * ts_m, :].rearrange(
                    "(p s) d -> p (s d)", p=P, s=ts_m
                )
                t_v = t[row0 : row0 + P * ts_m, :].rearrange(
                    "(p s) d -> p (s d)", p=P, s=ts_m
                )
                row0 += P * ts_m

                e_t = loads.tile(
                    [P, ts_m, D], mybir.dt.float8e4, name=f"e{m}", tag=f"e{m}"
                )
                t_t = loads.tile(
                    [P, ts_m, D], mybir.dt.float8e4, name=f"t{m}", tag=f"t{m}"
                )
                # f32 -> fp8e4 cast happens inside the SWDGE DMA
                nc.gpsimd.dma_start(out=e_t[:], in_=e_v)
                nc.gpsimd.dma_start(out=t_t[:], in_=t_v)

                # sub-tile-outer, chunk-inner: consecutive matmuls rotate
                # across PSUM banks, overlapping each matmul's drain with the
                # next one's fill (chunk-outer measured 6% slower).
                #
                # The width-1 sum-matmuls run every SECOND sub-tile in
                # DoubleRow mode (lhsT spans the sub-tile pair as
                # [128, 2, 128] with k-tile stride D): same math, half the
                # sum instructions.  DR costs ~127ns/instruction vs 2x50
                # plain, but the instruction stream shrinks ~2 pages of
                # 16KB, each of which steals ~0.9us of load bandwidth from
                # the fetch-hosting DMA engine.  Gram matmuls stay in plain
                # fp8 mode (DR measured slower per row for width-128).
                # (Measured dead end: also running the et cross-products in
                # DoubleRow saved one more fetch page but blew PE busy from
                # 69.6 to 83us -- interleaving plain and DR modes inside a
                # chunk stalls the PE pipeline far beyond the per-
                # instruction cost.  Only the subtile-pair-boundary DR
                # sum-matmuls below are cheap enough to mix in.)
                for s in range(ts_m):
                    first = m == 0 and s == 0
                    last = m == n_mega - 1 and s == ts_m - 1
                    unpaired = ts_m % 2 == 1 and s == ts_m - 1
                    # in the very last sub-tile, finish banks 2,3 first:
                    # their drain chain (two serial scalar copies + the
                    # scalar store DMA) is the longer tail pole, so giving
                    # scalar its data ~0.25us earlier shortens the end.
                    chunk_order = (2, 3, 0, 1) if last else range(N_CH)
                    for c in chunk_order:
                        ec = e_t[:, s, c * CH : (c + 1) * CH]
                        tc_ = t_t[:, s, c * CH : (c + 1) * CH]
                        ps = psums[c]
                        # start=True clears has_written for the whole bank, so
                        # only the very first matmul into the bank starts the
                        # group; later regions' first writes overwrite their
                        # (cleared) elements via per-element has_written.
                        nc.tensor.matmul(
                            ps[:, 0:CH], lhsT=ec, rhs=ec,
                            start=first, stop=False,
                        )
                        nc.tensor.matmul(
                            ps[:, CH : 2 * CH], lhsT=ec, rhs=tc_,
                            start=False, stop=False,
                        )
                        nc.tensor.matmul(
                            ps[:, 2 * CH : 3 * CH], lhsT=tc_, rhs=tc_,
                            start=False, stop=False,
                        )
                        if unpaired:
                            nc.tensor.matmul(
                                ps[:, 3 * CH : 3 * CH + 1], lhsT=ec, rhs=ones,
                                start=False, stop=False,
                            )
                            nc.tensor.matmul(
                                ps[:, 3 * CH + 1 : 3 * CH + 2], lhsT=tc_,
                                rhs=ones, start=False, stop=last,
                            )
                        elif s % 2 == 1:
                            ep = e_t[:, s - 1 : s + 1, c * CH : (c + 1) * CH]
                            tp = t_t[:, s - 1 : s + 1, c * CH : (c + 1) * CH]
                            nc.tensor.matmul(
                                ps[:, 3 * CH : 3 * CH + 1], lhsT=ep, rhs=ones2,
                                start=False, stop=False,
                                perf_mode=mybir.MatmulPerfMode.DoubleRow,
                            )
                            nc.tensor.matmul(
                                ps[:, 3 * CH + 1 : 3 * CH + 2], lhsT=tp,
                                rhs=ones2, start=False, stop=last,
                                perf_mode=mybir.MatmulPerfMode.DoubleRow,
                            )

            # drain the four chunk banks as soon as accumulation stops:
            # vector copies chunks 0-1 and scalar copies 2-3 (f32->f16) into
            # one [P, 4, 386] tile; each half leaves as its own DMA on its
            # own HWDGE queue (sync/scalar), so each store carries exactly
            # one producer wait and the two queues run in parallel with
            # contiguous 1544B-per-partition packets.
            obig = outs.tile([P, N_CH, SW], mybir.dt.float16, name="o", tag="o")
            nc.vector.tensor_copy(obig[:, 0, :], psums[0][:])
            nc.vector.tensor_copy(obig[:, 1, :], psums[1][:])
            nc.scalar.copy(obig[:, 2, :], psums[2][:])
            nc.scalar.copy(obig[:, 3, :], psums[3][:])
            nc.sync.dma_start(out=stats[:, 0:2, :], in_=obig[:, 0:2, :])
            nc.scalar.dma_start(out=stats[:, 2:4, :], in_=obig[:, 2:4, :])

    return nc


def _combine_host(per_core_stats):
    """per_core_stats: list of [128, N_CH, SW] f16 arrays -> f32 loss."""
    i = np.arange(CH)
    se = np.zeros(D, np.float64)
    st = np.zeros(D, np.float64)
    see = np.zeros(D, np.float64)
    stt = np.zeros(D, np.float64)
    set_ = np.zeros(D, np.float64)
    for g in per_core_stats:
        # [128, N_CH, SW] -> [N_CH, 128, SW]
        g = np.asarray(g, dtype=np.float64).transpose(1, 0, 2)
        see += g[:, i, i].reshape(D)
        set_ += g[:, i, CH + i].reshape(D)
        stt += g[:, i, 2 * CH + i].reshape(D)
        se += g[:, i, 3 * CH].reshape(D)
        st += g[:, i, 3 * CH + 1].reshape(D)

    me = se / B
    mt = st / B
    var_e = (see - B * me * me) / (B - 1)
    var_t = (stt - B * mt * mt) / (B - 1)
    std_e = np.sqrt(np.maximum(var_e, 0.0))
    std_t = np.sqrt(np.maximum(var_t, 0.0))
    cov = set_ - B * me * mt
    c_diag = cov / (B * (std_e + EPS) * (std_t + EPS))
    loss = np.sum((1.0 - c_diag) ** 2)
    return np.array(loss, dtype=np.float32)


def kernel(e, tau):
    global LAST_RESULT
    from concourse.bass_utils import run_bass_kernel_spmd

    e = np.ascontiguousarray(np.asarray(e, dtype=np.float32))
    tau = np.ascontiguousarray(np.asarray(tau, dtype=np.float32))
    assert e.shape == (B, D) and tau.shape == (B, D)

    if "nc" not in _nc_cache:
        _nc_cache["nc"] = _build()
    nc = _nc_cache["nc"]

    in_maps = [
        {"e": e[i * BS : (i + 1) * BS], "tau": tau[i * BS : (i + 1) * BS]}
        for i in range(N_CORES)
    ]
    stats = None
    err = None
    for _attempt in range(3):
        try:
            res = run_bass_kernel_spmd(
                nc, in_maps, core_ids=list(range(N_CORES)), trace=TRACE
            )
        except Exception as ex:  # transient runtime flake: retry
            err = ex
            continue
        LAST_RESULT = res
        stats = np.stack(
            [np.asarray(r["stats"], dtype=np.float32) for r in res.results]
        )
        # sums of <=8192 unit-scale terms stay far below 1e4 (f16 range);
        # anything else means a corrupted/raced execution -- rerun.
        if np.isfinite(stats).all() and np.abs(stats).max() < 1e8:
            break
    if stats is None:
        raise err
    return _combine_host(list(stats))



# revision 2
# speedup vs baseline: 1.1158x; 1.1158x over previous
"""Barlow Twins diagonal loss kernel for Trainium2 (8 NeuronCores).

Strategy (v2 — 2026-08-10 session)
----------------------------------
Data-parallel over batch: each core takes a 8192x512 shard of e/tau and
computes the five per-feature batch reductions (sum_e, sum_tau, sum_e2,
sum_tau2, sum_etau) with fp8 TensorEngine matmuls under a f32->fp8
SWDGE cast-DMA stream; the host combines the 8 partial stats in f64.

Measured structure of the exec window (profiled via ntff semaphores):
  * Stream: 33.55 MB f32 read/core.  Loads-only sustains ~410 GB/s
    (cast path, 16 SDMA engines); adding a ~1.5-2k-instruction Tensor
    program degrades it to ~350-390 (instruction fetch/dispatch +
    matmul LDW SBUF-read interference -- NONLINEAR in count, ~+7us for
    a 768-instr program with no data waits, ~+16us with waits).  Hence:
  * Combined mega tile [P, 2, ts, D] (half 0 = e, half 1 = t): same DMA
    packets as separate tiles, but ee and et merge into ONE 256-wide
    matmul whose rhs AP spans both halves -> Tensor program -26%%
    (1032 -> 776 matmuls), LDW SBUF reads -33%%.  Measured -9us.
  * MEGA_SCHED small-first + tapered tail: PE (~1.05-1.14us/subtile)
    is gated per-mega by the t-half load completion; small head mega
    starts PE early, tapered tail keeps the post-stream PE tail ~1.5us.
    (Uniform ts=8 DMA has an alternating 4.9/7.2us completion pathology
    on this platform; tapered/mixed schedules avoid it.)
  * Drain skips DMA proc-lane waits: loads are consumed by matmuls (PE
    lane progress implies load completion); the final stats store is NOT
    waited -- it completes inside the ~7.2us NEFF sem-zero epilogue that
    the walrus toolchain appends after our program (fixed cost, present
    even for a null kernel; measured via null/loads-only microbenches in
    bench.py).  Sem dirtiness from the unwaited store is neutralized by
    the program-start dma_reset+sem_clear of the whole kernel sem range.
  * Run-to-run variance on this fleet is +/-6us (ambient, interleaved
    A/B confirmed); compare configs pairwise, not across sessions.

All precision-critical accumulation is f32 PSUM; fp8 only quantizes the
products (~1e-5 relative on the final loss; measured 8.9e-06).
"""

import sys

if "/opt/trn_rl_repo" not in sys.path:
    sys.path.insert(0, "/opt/trn_rl_repo")

import numpy as np

N_CORES = 8
B, D = 65536, 512
BS = B // N_CORES
P = 128
CH = 128
N_CH = D // CH
SW = 3 * CH + 2
EPS = 1e-9

MEGA_SCHED = [2, 4, 8, 8, 8, 8, 8, 4, 4, 4, 2, 2, 1, 1]
N_LANES = 8
SKIP_DMASW_DRAIN = True
SKIP_DMAHW_DRAIN = True

TRACE = False
LAST_RESULT = None

_nc_cache = {}


def _build(bs=BS, sched=None):
    import concourse.bass as bass
    import concourse.tile as tile
    import concourse.tile_sem_assignment as tsa
    from concourse import mybir
    from concourse.tile_sem_assignment import PROC_NAME_TO_IDX

    tsa.NUM_SWDGE_GLOBAL_SEMS = N_LANES

    from concourse.vector_clock import ScopedClock, VectorClock

    skip_idx = set()
    if SKIP_DMASW_DRAIN:
        skip_idx |= {PROC_NAME_TO_IDX[f"DMASW{i}"] for i in range(8)}
    if SKIP_DMAHW_DRAIN:
        skip_idx |= {PROC_NAME_TO_IDX[f"DMAHW{i}"] for i in range(8)}

    class _SplitDrainTC(tile.TileContext):
        """One-wait-per-instruction drain (walrus single-wait-slot build),
        skipping DMA proc lanes whose completion is implied by their
        consumers (loads: consumed by PE) or covered by the program-start
        semaphore range-clear (stores, when SKIP_DMAHW_DRAIN)."""

        def _drain_and_barrier(self, tick_clock, wait_clock):
            gc = tick_clock.global_clock
            n = len(gc)
            for i in range(n):
                if gc[i] > 0 and i not in skip_idx:
                    vc = VectorClock([0] * n)
                    vc.require_at_least(i, gc[i])
                    nop = self.nc.sync.nop(nofuse=True)
                    wait_clock.add_sem_waits(nop.ins, ScopedClock({None: vc}))
            self.nc.sync.drain()
            self.nc.all_engine_barrier()
            assert self.sems is not None
            popped = self.nc._tile_sem_poison_stack.pop()
            assert popped is self._sem_poison
            self.nc.clear_and_free_semaphores(
                list(self.sems.allocated().values())
            )

    if sched is None:
        sched = list(MEGA_SCHED)
    assert sum(sched) * P == bs

    nc = bass.Bass()
    e = nc.dram_tensor("e", [bs, D], mybir.dt.float32, kind="ExternalInput")
    t = nc.dram_tensor("tau", [bs, D], mybir.dt.float32, kind="ExternalInput")
    stats = nc.dram_tensor(
        "stats", [P, N_CH, SW], mybir.dt.float16, kind="ExternalOutput"
    )

    with _SplitDrainTC(nc) as tc:
        with (
            tc.tile_pool(name="loads", bufs=1) as loads,
            tc.tile_pool(name="consts", bufs=1) as consts,
            tc.tile_pool(name="accs", bufs=1, space="PSUM") as accs,
            tc.tile_pool(name="outs", bufs=1) as outs,
        ):
            ones = consts.tile([P, 1], mybir.dt.float8e4)
            nc.vector.memset(ones, 1.0)
            ones2 = consts.tile([P, 2, 1], mybir.dt.float8e4)
            nc.vector.memset(ones2, 1.0)

            psums = [
                accs.tile([P, SW], mybir.dt.float32, name=f"acc{c}", tag=f"acc{c}")
                for c in range(N_CH)
            ]

            n_mega = len(sched)
            row0 = 0
            for m, ts_m in enumerate(sched):
                e_v = e[row0 : row0 + P * ts_m, :].rearrange(
                    "(p s) d -> p (s d)", p=P, s=ts_m
                )
                t_v = t[row0 : row0 + P * ts_m, :].rearrange(
                    "(p s) d -> p (s d)", p=P, s=ts_m
                )
                row0 += P * ts_m

                # combined tile: half 0 = e, half 1 = t
                et_t = loads.tile(
                    [P, 2, ts_m, D], mybir.dt.float8e4, name=f"et{m}", tag=f"et{m}"
                )
                nc.gpsimd.dma_start(out=et_t[:, 0], in_=e_v)
                nc.gpsimd.dma_start(out=et_t[:, 1], in_=t_v)

                for s in range(ts_m):
                    first = m == 0 and s == 0
                    last = m == n_mega - 1 and s == ts_m - 1
                    unpaired = ts_m % 2 == 1 and s == ts_m - 1
                    chunk_order = (2, 3, 0, 1) if last else range(N_CH)
                    for c in chunk_order:
                        ec = et_t[:, 0, s, c * CH : (c + 1) * CH]
                        tc_ = et_t[:, 1, s, c * CH : (c + 1) * CH]
                        both = et_t[:, :, s, c * CH : (c + 1) * CH]
                        ps = psums[c]
                        # [ee | et] in one 256-wide matmul (rhs spans halves)
                        nc.tensor.matmul(
                            ps[:, 0 : 2 * CH], lhsT=ec, rhs=both,
                            start=first, stop=False,
                        )
                        nc.tensor.matmul(
                            ps[:, 2 * CH : 3 * CH], lhsT=tc_, rhs=tc_,
                            start=False, stop=False,
                        )
                        if unpaired:
                            nc.tensor.matmul(
                                ps[:, 3 * CH : 3 * CH + 1], lhsT=ec, rhs=ones,
                                start=False, stop=False,
                            )
                            nc.tensor.matmul(
                                ps[:, 3 * CH + 1 : 3 * CH + 2], lhsT=tc_,
                                rhs=ones, start=False, stop=last,
                            )
                        elif s % 2 == 1:
                            ep = et_t[:, 0, s - 1 : s + 1, c * CH : (c + 1) * CH]
                            tp = et_t[:, 1, s - 1 : s + 1, c * CH : (c + 1) * CH]
                            nc.tensor.matmul(
                                ps[:, 3 * CH : 3 * CH + 1], lhsT=ep, rhs=ones2,
                                start=False, stop=False,
                                perf_mode=mybir.MatmulPerfMode.DoubleRow,
                            )
                            nc.tensor.matmul(
                                ps[:, 3 * CH + 1 : 3 * CH + 2], lhsT=tp,
                                rhs=ones2, start=False, stop=last,
                                perf_mode=mybir.MatmulPerfMode.DoubleRow,
                            )

            obig = outs.tile([P, N_CH, SW], mybir.dt.float16, name="o", tag="o")
            nc.vector.tensor_copy(obig[:, 0, :], psums[0][:])
            nc.vector.tensor_copy(obig[:, 1, :], psums[1][:])
            nc.scalar.copy(obig[:, 2, :], psums[2][:])
            nc.scalar.copy(obig[:, 3, :], psums[3][:])
            nc.sync.dma_start(out=stats[:, 0:2, :], in_=obig[:, 0:2, :])
            nc.scalar.dma_start(out=stats[:, 2:4, :], in_=obig[:, 2:4, :])

    return nc


def _combine_host(per_core_stats):
    i = np.arange(CH)
    se = np.zeros(D, np.float64)
    st = np.zeros(D, np.float64)
    see = np.zeros(D, np.float64)
    stt = np.zeros(D, np.float64)
    set_ = np.zeros(D, np.float64)
    for g in per_core_stats:
        g = np.asarray(g, dtype=np.float64).transpose(1, 0, 2)
        see += g[:, i, i].reshape(D)
        set_ += g[:, i, CH + i].reshape(D)
        stt += g[:, i, 2 * CH + i].reshape(D)
        se += g[:, i, 3 * CH].reshape(D)
        st += g[:, i, 3 * CH + 1].reshape(D)

    me = se / B
    mt = st / B
    var_e = (see - B * me * me) / (B - 1)
    var_t = (stt - B * mt * mt) / (B - 1)
    std_e = np.sqrt(np.maximum(var_e, 0.0))
    std_t = np.sqrt(np.maximum(var_t, 0.0))
    cov = set_ - B * me * mt
    c_diag = cov / (B * (std_e + EPS) * (std_t + EPS))
    loss = np.sum((1.0 - c_diag) ** 2)
    return np.array(loss, dtype=np.float32)


def kernel(e, tau):
    global LAST_RESULT
    from concourse.bass_utils import run_bass_kernel_spmd

    e = np.ascontiguousarray(np.asarray(e, dtype=np.float32))
    tau = np.ascontiguousarray(np.asarray(tau, dtype=np.float32))
    assert e.shape == (B, D) and tau.shape == (B, D)

    if "nc" not in _nc_cache:
        _nc_cache["nc"] = _build()
    nc = _nc_cache["nc"]

    in_maps = [
        {"e": e[i * BS : (i + 1) * BS], "tau": tau[i * BS : (i + 1) * BS]}
        for i in range(N_CORES)
    ]
    stats = None
    err = None
    for _attempt in range(3):
        try:
            res = run_bass_kernel_spmd(
                nc, in_maps, core_ids=list(range(N_CORES)), trace=TRACE
            )
        except Exception as ex:
            err = ex
            continue
        LAST_RESULT = res
        stats = np.stack(
            [np.asarray(r["stats"], dtype=np.float32) for r in res.results]
        )
        if np.isfinite(stats).all() and np.abs(stats).max() < 1e8:
            break
    if stats is None:
        raise err
    return _combine_host(list(stats))
